# revision 14
# baseline (speedup 1.0000x reference)
"""Trainium2 Bass kernel for nn_Model_29592324670139 (dense transformer).

Sharding: 8 cores = 4 pairs. Pair b handles batch item b; within a pair the
672-token sequence (21 vars x 32 windows, window-major order) is split by
window parity (rank0 = even windows, rank1 = odd windows), 336 tokens each.
Per layer, each core projects Q/K/V for its tokens; K/V are AllGathered
within the pair; attention/FFN/LN run on local tokens. The final pooled
feature sum is AllGathered, and head+MLP run redundantly per pair.

Activations are feature-major ([d, token] with d on partitions). Matmuls in
bf16 with fp32 PSUM accumulation; softmax/LN statistics in fp32.

Self-contained: hardcodes all shapes; only needs numpy/ml_dtypes/concourse.
"""

import numpy as np
import ml_dtypes

import concourse.bass as bass
import concourse.tile as tile
from concourse import bacc, mybir
from concourse.bass import ts, ds
from concourse.bass_utils import run_bass_kernel_spmd

F32 = mybir.dt.float32
BF16 = mybir.dt.bfloat16
F32R = mybir.dt.float32r
AX = mybir.AluOpType
AF = mybir.ActivationFunctionType
XL = mybir.AxisListType

B, L, C = 4, 3072, 21
P, OUT, D, H, NL, DFF = 96, 96, 1024, 16, 2, 4096
NW = 32          # windows
SL = 336         # local tokens per core
S = 672          # full sequence
HD = 64          # head dim
NKC = D // 128   # 8 k-chunks of d_model
NFC = DFF // 128  # 32 chunks of d_ff

REPLICA_GROUPS = [[0, 1], [2, 3], [4, 5], [6, 7]]

_BUILT = None  # cached (nc, input_names)

LAST_RESULT = None  # stash of the last BassKernelResults (for test harness)


# ----------------------------------------------------------------------------
# device program
# ----------------------------------------------------------------------------

def _build():
    nc = bacc.Bacc("TRN2", target_bir_lowering=False, debug=False,
                   enable_asserts=False, num_devices=8)

    t = {}

    def din(name, shape, dt):
        t[name] = nc.dram_tensor(name, list(shape), dt, kind="ExternalInput").ap()

    # per-core data
    din("xfull", (C, L), F32)
    din("xloc", (P, SL), BF16)
    din("maskM", (112, 6, SL), BF16)
    # embedding
    din("embW", (P, D), BF16)
    din("embWsumN", (128, NKC), F32)
    din("embB", (128, NKC), F32)
    for l in range(NL):
        for w, bvec in (("Wq", "bq"), ("Wk", "bk"), ("Wo", "bo")):
            din(f"{w}{l}", (128, NKC, D), BF16)      # [p, kc, n] pre-arranged
            din(f"{bvec}{l}", (128, NKC), F32)
        din(f"Wv{l}", (128, NKC, D), BF16)
        din(f"bv{l}", (1, D), F32)
        din(f"W1{l}", (4, 128, NKC, 1024), BF16)     # quarters [j, p, kc, n]
        din(f"b1{l}", (128, NFC), F32)
        din(f"W2{l}", (NKC, 128, NFC, 128), BF16)    # per-oc [oc, p, kc, c]
        din(f"b2{l}", (128, NKC), F32)
        for v in ("ln1s", "ln1b", "ln2s", "ln2b"):
            din(f"{v}{l}", (128, NKC), F32)
    din("lnfs", (128, NKC), F32)
    din("lnfb", (128, NKC), F32)
    din("headW", (128, NKC, OUT), BF16)
    din("headB", (OUT, 1), F32)
    din("c1W", (OUT, 256), BF16)
    din("c1B", (128, 2), F32)
    din("c2W", (128, 2, 64), BF16)
    din("c2B", (64, 1), F32)
    din("c3W", (64, 2), BF16)
    din("c3B", (2, 1), F32)

    out_dram = nc.dram_tensor("out", [2, 1], F32, kind="ExternalOutput").ap()

    with tile.TileContext(nc) as tc:
        _emit(tc, t, out_dram)

    nc.compile()
    return nc, set(t.keys())


def _emit(tc, t, out_dram):
    from contextlib import ExitStack
    nc = tc.nc
    ctx = ExitStack()

    # ---------------- pools ----------------
    constp = ctx.enter_context(tc.tile_pool(name="constp", bufs=1))
    wpool = ctx.enter_context(tc.tile_pool(name="wpool", bufs=2))
    actp = ctx.enter_context(tc.tile_pool(name="actp", bufs=1))
    esbp = ctx.enter_context(tc.tile_pool(name="esbp", bufs=2))
    lnp = ctx.enter_context(tc.tile_pool(name="lnp", bufs=1))
    recp = ctx.enter_context(tc.tile_pool(name="recp", bufs=2))
    w2p = ctx.enter_context(tc.tile_pool(name="w2p", bufs=2))
    bcp = ctx.enter_context(tc.tile_pool(name="bcp", bufs=1))
    dramp = ctx.enter_context(tc.tile_pool(name="dramp", bufs=1, space="DRAM"))
    psS = ctx.enter_context(tc.tile_pool(name="psS", bufs=2, space="PSUM"))
    psAV = ctx.enter_context(tc.tile_pool(name="psAV", bufs=2, space="PSUM"))
    psMM = ctx.enter_context(tc.tile_pool(name="psMM", bufs=2, space="PSUM"))
    psST = ctx.enter_context(tc.tile_pool(name="psST", bufs=2, space="PSUM"))

    def single(shape, dt, name, **kw):
        tl, free = tc.tile(shape, dt, name=name, **kw)
        ctx.callback(free)
        return tl

    # ---------------- constants / small tensors ----------------
    def vec_to_sb(name, dim):
        # pre-arranged [128, dim//128] f32 dram -> sbuf
        nchunk = dim // 128
        sb = constp.tile([128, nchunk], F32, name=f"sb_{name}", tag=f"sb_{name}")
        nc.sync.dma_start(out=sb[:], in_=t[name][:])
        return sb

    sb_embWsumN = vec_to_sb("embWsumN", D)
    sb_embB = vec_to_sb("embB", D)
    bias_sb = {}
    for l in range(NL):
        for v in ("bq", "bk", "bo", "b2", "ln1s", "ln1b", "ln2s", "ln2b"):
            bias_sb[f"{v}{l}"] = vec_to_sb(f"{v}{l}", D)
        bias_sb[f"b1{l}"] = vec_to_sb(f"b1{l}", DFF)
    sb_lnfs = vec_to_sb("lnfs", D)
    sb_lnfb = vec_to_sb("lnfb", D)

    # bv as [1, 1024] rows for partition_broadcast
    sb_bv = {}
    for l in range(NL):
        bv1 = constp.tile([1, D], F32, name=f"bv1_{l}", tag=f"bv1_{l}")
        nc.sync.dma_start(out=bv1[:], in_=t[f"bv{l}"][:])
        sb_bv[l] = bv1

    sb_headB = constp.tile([OUT, 1], F32, name="sb_headB", tag="sb_headB")
    nc.sync.dma_start(out=sb_headB[:], in_=t["headB"][:])
    sb_c1B = constp.tile([128, 2], F32, name="sb_c1B", tag="sb_c1B")
    nc.sync.dma_start(out=sb_c1B[:], in_=t["c1B"][:])
    sb_c2B = constp.tile([64, 1], F32, name="sb_c2B", tag="sb_c2B")
    nc.sync.dma_start(out=sb_c2B[:], in_=t["c2B"][:])
    sb_c3B = constp.tile([2, 1], F32, name="sb_c3B", tag="sb_c3B")
    nc.sync.dma_start(out=sb_c3B[:], in_=t["c3B"][:])

    sb_c1W = constp.tile([OUT, 256], BF16, name="sb_c1W", tag="sb_c1W")
    nc.sync.dma_start(out=sb_c1W[:], in_=t["c1W"][:])
    sb_c2W = constp.tile([128, 2, 64], BF16, name="sb_c2W", tag="sb_c2W")
    nc.sync.dma_start(out=sb_c2W[:], in_=t["c2W"][:])
    sb_c3W = constp.tile([64, 2], BF16, name="sb_c3W", tag="sb_c3W")
    nc.sync.dma_start(out=sb_c3W[:], in_=t["c3W"][:])
    sb_headW = constp.tile([128, NKC, OUT], BF16, name="sb_headW", tag="sb_headW")
    nc.sync.dma_start(out=sb_headW[:], in_=t["headW"][:])

    sb_mask = constp.tile([112, 6, SL], BF16, name="sb_mask", tag="sb_mask")
    nc.sync.dma_start(out=sb_mask[:], in_=t["maskM"][:])

    ones_bf = constp.tile([128, 1], BF16, name="ones_bf", tag="ones_bf")
    nc.vector.memset(ones_bf[:], 1.0)
    eps6_sb = constp.tile([C, 1], F32, name="eps6_sb", tag="eps6_sb")
    nc.vector.memset(eps6_sb[:], 1e-6)
    eps5_sb = constp.tile([1, 1], F32, name="eps5_sb", tag="eps5_sb")
    nc.vector.memset(eps5_sb[:], 1e-5)
    zero_sb = constp.tile([128, 1], F32, name="zero_sb", tag="zero_sb")
    nc.vector.memset(zero_sb[:], 0.0)

    # ---------------- stage 0: instance norm stats ----------------
    st6 = constp.tile([C, 6, 6], F32, name="st6", tag="st6")
    for i in range(6):
        xfc = recp.tile([C, 512], F32, name="xfc", tag="xfc")
        nc.sync.dma_start(out=xfc[:], in_=t["xfull"][:, ts(i, 512)])
        nc.vector.bn_stats(out=st6[:, i, :], in_=xfc[:])
    mv = constp.tile([C, 2], F32, name="mv", tag="mv")
    nc.vector.bn_aggr(out=mv[:], in_=st6[:])
    std21 = constp.tile([C, 1], F32, name="std21", tag="std21")
    nc.scalar.activation(out=std21[:], in_=mv[:, 1:2], func=AF.Sqrt, bias=eps6_sb[:])
    stat2 = constp.tile([C, 2], F32, name="stat2", tag="stat2")
    nc.vector.reciprocal(out=stat2[:, 0:1], in_=std21[:])
    nc.vector.tensor_mul(stat2[:, 1:2], mv[:, 0:1], stat2[:, 0:1])

    stat_dram = dramp.tile([C, 2], F32, name="stat_dram", tag="stat_dram")
    nc.sync.dma_start(out=stat_dram[:], in_=stat2[:])
    # per-token [1, 336] vectors: token t -> channel c = t % 21
    rstd_tok = constp.tile([1, SL], F32, name="rstd_tok", tag="rstd_tok")
    nc.sync.dma_start(
        out=rstd_tok[:].rearrange("p (n c) -> p n c", c=C),
        in_=bass.AP(tensor=stat_dram[:].tensor, offset=stat_dram[:].offset,
                    ap=[[0, 16], [2, C]]))
    mrs_tok = constp.tile([1, SL], F32, name="mrs_tok", tag="mrs_tok")
    nc.sync.dma_start(
        out=mrs_tok[:].rearrange("p (n c) -> p n c", c=C),
        in_=bass.AP(tensor=stat_dram[:].tensor, offset=stat_dram[:].offset + 1,
                    ap=[[0, 16], [2, C]]))
    rt_b = constp.tile([128, SL], F32, name="rt_b", tag="rt_b")
    nc.gpsimd.partition_broadcast(out_ap=rt_b[:], in_ap=rstd_tok[:])
    mrs_b = constp.tile([128, SL], F32, name="mrs_b", tag="mrs_b")
    nc.gpsimd.partition_broadcast(out_ap=mrs_b[:], in_ap=mrs_tok[:])

    # ---------------- stage 1: embedding ----------------
    xloc_sb = constp.tile([P, SL], BF16, name="xloc_sb", tag="xloc_sb")
    nc.sync.dma_start(out=xloc_sb[:], in_=t["xloc"][:])
    embW_sb = constp.tile([P, D], BF16, name="embW_sb", tag="embW_sb")
    nc.sync.dma_start(out=embW_sb[:], in_=t["embW"][:])

    h_f32 = single([128, NKC, SL], F32, "h_f32")
    h_bf = single([128, NKC, SL], BF16, "h_bf")
    d1 = single([128, NKC, SL], F32, "d1")
    sq_bf = single([128, NKC, SL], BF16, "sq_bf")

    for c8 in range(NKC):
        pse = psMM.tile([128, SL], F32, name="pse", tag="mm")
        nc.tensor.matmul(pse[:], lhsT=embW_sb[:, ts(c8, 128)], rhs=xloc_sb[:],
                         start=True, stop=True)
        nc.vector.tensor_mul(d1[:, c8, :], pse[:], rt_b[:])
        nc.vector.scalar_tensor_tensor(
            out=d1[:, c8, :], in0=mrs_b[:], scalar=sb_embWsumN[:, c8:c8 + 1],
            in1=d1[:, c8, :], op0=AX.mult, op1=AX.add)
        nc.scalar.activation(out=h_f32[:, c8, :], in_=d1[:, c8, :],
                             func=AF.Identity, bias=sb_embB[:, c8:c8 + 1])
        nc.scalar.activation(out=h_bf[:, c8, :], in_=d1[:, c8, :],
                             func=AF.Identity, bias=sb_embB[:, c8:c8 + 1])

    # ---------------- helpers ----------------
    def load_w(src, n, tag="w"):
        """Load a pre-arranged [128, NKC, n] bf16 weight tile."""
        w = wpool.tile([128, NKC, n], BF16, name="w_t", tag=tag)
        nc.sync.dma_start(out=w[:], in_=src)
        return w

    def ln(s_sb, b_sb, write_bf, src):
        """Feature-major layernorm: reads src (f32), writes h_f32 (+h_bf)."""
        ps_sum = psST.tile([1, SL], F32, name="ps_sum", tag="st")
        ps_sq = psST.tile([1, SL], F32, name="ps_sq", tag="st")
        for c8 in range(NKC):
            nc.scalar.activation(out=h_bf[:, c8, :], in_=src[:, c8, :],
                                 func=AF.Identity, bias=zero_sb[:])
            nc.scalar.activation(out=sq_bf[:, c8, :], in_=src[:, c8, :],
                                 func=AF.Square)
        for c8 in range(NKC):
            nc.tensor.matmul(ps_sum[:], lhsT=ones_bf[:],
                             rhs=h_bf[:, c8, :],
                             start=(c8 == 0), stop=(c8 == NKC - 1))
        for c8 in range(NKC):
            nc.tensor.matmul(ps_sq[:], lhsT=ones_bf[:],
                             rhs=sq_bf[:, c8, :],
                             start=(c8 == 0), stop=(c8 == NKC - 1))
        mean1 = lnp.tile([1, SL], F32, name="mean1", tag="mean1")
        nc.scalar.activation(out=mean1[:], in_=ps_sum[:], func=AF.Copy,
                             scale=1.0 / D)
        ms1 = lnp.tile([1, SL], F32, name="ms1", tag="ms1")
        nc.vector.tensor_mul(ms1[:], mean1[:], mean1[:])
        var1 = lnp.tile([1, SL], F32, name="var1", tag="var1")
        nc.vector.scalar_tensor_tensor(out=var1[:], in0=ps_sq[:], scalar=1.0 / D,
                                       in1=ms1[:], op0=AX.mult, op1=AX.subtract)
        std1 = lnp.tile([1, SL], F32, name="std1", tag="std1")
        nc.scalar.activation(out=std1[:], in_=var1[:], func=AF.Sqrt, bias=eps5_sb[:])
        r1 = lnp.tile([1, SL], F32, name="r1", tag="r1")
        nc.vector.reciprocal(out=r1[:], in_=std1[:])
        mean_b = bcp.tile([128, SL], F32, name="mean_b", tag="mean_b")
        nc.gpsimd.partition_broadcast(out_ap=mean_b[:], in_ap=mean1[:])
        r_b = bcp.tile([128, SL], F32, name="r_b", tag="r_b")
        nc.gpsimd.partition_broadcast(out_ap=r_b[:], in_ap=r1[:])
        for c8 in range(NKC):
            nc.vector.tensor_sub(d1[:, c8, :], src[:, c8, :], mean_b[:])
            nc.vector.scalar_tensor_tensor(
                out=d1[:, c8, :], in0=d1[:, c8, :], scalar=s_sb[:, c8:c8 + 1],
                in1=r_b[:], op0=AX.mult, op1=AX.mult)
            nc.scalar.activation(out=h_f32[:, c8, :], in_=d1[:, c8, :],
                                 func=AF.Identity, bias=b_sb[:, c8:c8 + 1])
            if write_bf:
                nc.scalar.activation(out=h_bf[:, c8, :], in_=d1[:, c8, :],
                                     func=AF.Identity, bias=b_sb[:, c8:c8 + 1])

    # ---------------- transformer layers ----------------
    for l in range(NL):
        wq = load_w(t[f"Wq{l}"][:], D)
        wk = load_w(t[f"Wk{l}"][:], D)
        wv = load_w(t[f"Wv{l}"][:], D)

        # Q, K feature-major projections -> bf16
        q_sb = actp.tile([128, NKC, SL], BF16, name="q_sb", tag="q_sb")
        k_sb = actp.tile([128, NKC, SL], BF16, name="k_sb", tag="k_sb")
        for dst, w, bn in ((q_sb, wq, f"bq{l}"), (k_sb, wk, f"bk{l}")):
            bsb = bias_sb[bn]
            for oc in range(NKC):
                psp = psMM.tile([128, SL], F32, name="psp", tag="mm")
                for kc in range(NKC):
                    nc.tensor.matmul(psp[:], lhsT=w[:, kc, ts(oc, 128)],
                                     rhs=h_bf[:, kc, :],
                                     start=(kc == 0), stop=(kc == NKC - 1))
                nc.scalar.activation(out=dst[:, oc, :], in_=psp[:],
                                     func=AF.Identity, bias=bsb[:, oc:oc + 1])

        # V token-major projection  [112, 3, 1024]
        bv_b = actp.tile([112, D], F32, name="bv_b", tag="bv_b")
        nc.gpsimd.partition_broadcast(out_ap=bv_b[:], in_ap=sb_bv[l][:])
        v_sb = actp.tile([112, 3, D], BF16, name="v_sb", tag="v_sb")
        for tc3 in range(3):
            for nh in range(2):
                psv = psMM.tile([112, 512], F32, name="psv", tag="mm")
                for kc in range(NKC):
                    nc.tensor.matmul(psv[:], lhsT=h_bf[:, kc, ds(tc3 * 112, 112)],
                                     rhs=wv[:, kc, ts(nh, 512)],
                                     start=(kc == 0), stop=(kc == NKC - 1))
                nc.vector.tensor_add(
                    v_sb[:, tc3, ts(nh, 512)], psv[:], bv_b[:, ts(nh, 512)])

        # bounce K,V to DRAM (flat: K region then V region) and AllGather
        KREG = 128 * NKC * SL
        VREG = SL * D
        bnc_in = dramp.tile([KREG + VREG], BF16, name=f"bnc_in{l}",
                            tag=f"bnc_in{l}")
        nc.sync.dma_start(
            out=bnc_in[ds(0, KREG)].rearrange("(kc p tk) -> p kc tk",
                                              p=128, tk=SL),
            in_=k_sb[:])
        nc.sync.dma_start(
            out=bnc_in[ds(KREG, VREG)].rearrange("(t3 p he) -> p t3 he",
                                                 p=112, he=D),
            in_=v_sb[:])
        bnc_out = single([2 * (KREG + VREG)], BF16, f"bnc_out{l}", space="DRAM",
                         addr_space="Shared")
        nc.gpsimd.collective_compute(
            "AllGather", AX.bypass, replica_groups=REPLICA_GROUPS,
            ins=[bnc_in[:]], outs=[bnc_out[:]])

        k_full = actp.tile([128, NKC, S], BF16, name="k_full", tag="k_full")
        for r in range(2):
            nc.sync.dma_start(
                out=k_full[:, :, ds(r * SL, SL)],
                in_=bnc_out[ds(r * (KREG + VREG), KREG)].rearrange(
                    "(kc p tk) -> p kc tk", p=128, tk=SL))
        v_full = actp.tile([112, 6, H, HD + 1], BF16, name="v_full", tag="v_full")
        nc.vector.memset(v_full[:, :, :, HD:HD + 1], 1.0)
        for c6 in range(6):
            off = (c6 // 3) * (KREG + VREG) + KREG + (c6 % 3) * 112 * D
            nc.sync.dma_start(
                out=v_full[:, c6, :, 0:HD],
                in_=bnc_out[ds(off, 112 * D)].rearrange(
                    "(p hh e) -> p hh e", p=112, e=HD))

        # attention
        att_sb = actp.tile([128, NKC, SL], BF16, name="att_sb", tag="att_sb")
        stage_odd = actp.tile([64, NKC, SL], BF16, name="stage_odd", tag="stage_odd")
        for hh in range(H):
            hb2 = 64 * (hh % 2)
            hc = hh // 2
            esb = esbp.tile([112, 6, SL], BF16, name="esb", tag="esb")
            for cc in range(6):
                pss = psS.tile([112, SL], F32, name="pss", tag="s")
                nc.tensor.matmul(pss[:],
                                 lhsT=k_full[ds(hb2, 64), hc, ts(cc, 112)],
                                 rhs=q_sb[ds(hb2, 64), hc, :],
                                 start=True, stop=True)
                nc.scalar.activation(out=esb[:, cc, :], in_=pss[:], func=AF.Exp)
                nc.vector.tensor_mul(esb[:, cc, :], esb[:, cc, :], sb_mask[:, cc, :])
            psa = psAV.tile([HD + 1, SL], F32, name="psa", tag="av")
            for cc in range(6):
                nc.tensor.matmul(psa[:], lhsT=v_full[:, cc, hh, :],
                                 rhs=esb[:, cc, :],
                                 start=(cc == 0), stop=(cc == 5))
            rec = recp.tile([1, SL], F32, name="rec", tag="rec")
            nc.vector.reciprocal(out=rec[:], in_=psa[ds(HD, 1), :])
            rb = recp.tile([64, SL], F32, name="rb", tag="rb")
            nc.gpsimd.partition_broadcast(out_ap=rb[:], in_ap=rec[:])
            dst = att_sb[0:64, hc, :] if hh % 2 == 0 else stage_odd[:, hc, :]
            nc.vector.tensor_mul(dst, psa[0:HD, :], rb[:])
        nc.sync.dma_start(out=att_sb[ds(64, 64), :, :], in_=stage_odd[:])

        # out-proj + residual
        wo = load_w(t[f"Wo{l}"][:], D)
        bo_sb = bias_sb[f"bo{l}"]
        for oc in range(NKC):
            pso = psMM.tile([128, SL], F32, name="pso", tag="mm")
            for kc in range(NKC):
                nc.tensor.matmul(pso[:], lhsT=wo[:, kc, ts(oc, 128)],
                                 rhs=att_sb[:, kc, :],
                                 start=(kc == 0), stop=(kc == NKC - 1))
            nc.vector.scalar_tensor_tensor(
                out=d1[:, oc, :], in0=pso[:], scalar=bo_sb[:, oc:oc + 1],
                in1=h_f32[:, oc, :], op0=AX.add, op1=AX.add)

        ln(bias_sb[f"ln1s{l}"], bias_sb[f"ln1b{l}"], write_bf=True, src=d1)

        # FFN
        w1q = [load_w(t[f"W1{l}"][j], 1024) for j in range(4)]
        g_sb = actp.tile([128, NFC, SL], BF16, name="g_sb", tag="g_sb")
        b1_sb = bias_sb[f"b1{l}"]
        for fc in range(NFC):
            w1 = w1q[fc // 8]
            psf = psMM.tile([128, SL], F32, name="psf", tag="mm")
            for kc in range(NKC):
                nc.tensor.matmul(psf[:], lhsT=w1[:, kc, ts(fc % 8, 128)],
                                 rhs=h_bf[:, kc, :],
                                 start=(kc == 0), stop=(kc == NKC - 1))
            nc.scalar.activation(out=g_sb[:, fc, :], in_=psf[:], func=AF.Gelu,
                                 bias=b1_sb[:, fc:fc + 1])
        b2_sb = bias_sb[f"b2{l}"]
        for oc in range(NKC):
            w2 = w2p.tile([128, NFC, 128], BF16, name="w2oc", tag="w2oc")
            nc.sync.dma_start(out=w2[:], in_=t[f"W2{l}"][oc])
            psy = psMM.tile([128, SL], F32, name="psy", tag="mm")
            for kc in range(NFC):
                nc.tensor.matmul(psy[:], lhsT=w2[:, kc, ts(0, 128)],
                                 rhs=g_sb[:, kc, :],
                                 start=(kc == 0), stop=(kc == NFC - 1))
            nc.vector.scalar_tensor_tensor(
                out=d1[:, oc, :], in0=psy[:], scalar=b2_sb[:, oc:oc + 1],
                in1=h_f32[:, oc, :], op0=AX.add, op1=AX.add)

        ln(bias_sb[f"ln2s{l}"], bias_sb[f"ln2b{l}"], write_bf=(l < NL - 1), src=d1)

    # ---------------- final: LN_f, pooled head, MLP ----------------
    ln(sb_lnfs, sb_lnfb, write_bf=False, src=h_f32)
    hsum = constp.tile([128, NKC], F32, name="hsum", tag="hsum")
    for c8 in range(NKC):
        nc.vector.reduce_sum(out=hsum[:, c8:c8 + 1], in_=h_f32[:, c8, :], axis=XL.X)
    fin_in = dramp.tile([D], F32, name="fin_in", tag="fin_in")
    nc.sync.dma_start(out=fin_in[:].rearrange("(kc p) -> p kc", p=128), in_=hsum[:])
    fin_out = single([2 * D], F32, "fin_out", space="DRAM",
                     addr_space="Shared")
    nc.gpsimd.collective_compute(
        "AllGather", AX.bypass, replica_groups=REPLICA_GROUPS,
        ins=[fin_in[:]], outs=[fin_out[:]])
    ffx = constp.tile([128, NKC, 2], F32, name="ffx", tag="ffx")
    for r in range(2):
        nc.sync.dma_start(
            out=ffx[:, :, r],
            in_=fin_out[ds(r * D, D)].rearrange("(kc p) -> p kc", p=128))
    hb2t = constp.tile([128, NKC], F32, name="hb2t", tag="hb2t")
    nc.vector.tensor_add(hb2t[:], ffx[:, :, 0], ffx[:, :, 1])
    hbar_bf = constp.tile([128, NKC], BF16, name="hbar_bf", tag="hbar_bf")
    nc.scalar.activation(out=hbar_bf[:], in_=hb2t[:], func=AF.Copy, scale=1.0 / S)

    psh = psMM.tile([OUT, 1], F32, name="psh", tag="mm")
    for kc in range(NKC):
        nc.tensor.matmul(psh[:], lhsT=sb_headW[:, kc, :],
                         rhs=hbar_bf[:, kc:kc + 1],
                         start=(kc == 0), stop=(kc == NKC - 1))
    feat_bf = constp.tile([OUT, 1], BF16, name="feat_bf", tag="feat_bf")
    nc.scalar.activation(out=feat_bf[:], in_=psh[:], func=AF.Identity,
                         bias=sb_headB[:])

    z1_bf = constp.tile([128, 2], BF16, name="z1_bf", tag="z1_bf")
    for i2 in range(2):
        psc = psMM.tile([128, 1], F32, name="psc", tag="mm")
        nc.tensor.matmul(psc[:], lhsT=sb_c1W[:, ts(i2, 128)], rhs=feat_bf[:],
                         start=True, stop=True)
        nc.scalar.activation(out=z1_bf[:, i2:i2 + 1], in_=psc[:], func=AF.Relu,
                             bias=sb_c1B[:, i2:i2 + 1])
    psc2 = psMM.tile([64, 1], F32, name="psc2", tag="mm")
    for kc in range(2):
        nc.tensor.matmul(psc2[:], lhsT=sb_c2W[:, kc, :], rhs=z1_bf[:, kc:kc + 1],
                         start=(kc == 0), stop=(kc == 1))
    z2_bf = constp.tile([64, 1], BF16, name="z2_bf", tag="z2_bf")
    nc.scalar.activation(out=z2_bf[:], in_=psc2[:], func=AF.Relu, bias=sb_c2B[:])
    psc3 = psMM.tile([2, 1], F32, name="psc3", tag="mm")
    nc.tensor.matmul(psc3[:], lhsT=sb_c3W[:], rhs=z2_bf[:], start=True, stop=True)
    out_sb = constp.tile([2, 1], F32, name="out_sb", tag="out_sb")
    nc.scalar.activation(out=out_sb[:], in_=psc3[:], func=AF.Identity,
                         bias=sb_c3B[:])
    nc.sync.dma_start(out=out_dram[:], in_=out_sb[:])
    ctx.close()


# ----------------------------------------------------------------------------
# host side
# ----------------------------------------------------------------------------

def _bf16(x):
    return np.ascontiguousarray(np.asarray(x, dtype=np.float32)).astype(
        ml_dtypes.bfloat16)


def _f32(x):
    return np.ascontiguousarray(np.asarray(x, dtype=np.float32))


def _wtile(a):
    # [D_in, N] -> [128, D_in//128, N] (p, kc, n)
    a = np.asarray(a, np.float32)
    din, n = a.shape
    return _bf16(a.reshape(din // 128, 128, n).transpose(1, 0, 2))


def _btile(a, p=128):
    # [dim] -> [p, dim//p]
    a = np.asarray(a, np.float32)
    return _f32(a.reshape(-1, p).T)


def _host_weights(inp):
    w = {}
    w["embW"] = _bf16(inp["emb_W"])
    w["embWsumN"] = _btile(-np.asarray(inp["emb_W"], np.float32).sum(0))
    w["embB"] = _btile(inp["emb_b"])
    for l in range(NL):
        w[f"Wq{l}"] = _wtile(np.asarray(inp["Wq"][l], np.float32) * 0.125)
        w[f"bq{l}"] = _btile(np.asarray(inp["bq"][l], np.float32) * 0.125)
        w[f"Wk{l}"] = _wtile(inp["Wk"][l])
        w[f"bk{l}"] = _btile(inp["bk"][l])
        w[f"Wv{l}"] = _wtile(inp["Wv"][l])
        w[f"bv{l}"] = _f32(np.asarray(inp["bv"][l], np.float32)[None, :])
        w[f"Wo{l}"] = _wtile(inp["Wo"][l])
        w[f"bo{l}"] = _btile(inp["bo"][l])
        w1 = np.asarray(inp["W1"][l], np.float32)
        w[f"W1{l}"] = _bf16(w1.reshape(NKC, 128, 4, 1024).transpose(2, 1, 0, 3))
        w[f"b1{l}"] = _btile(inp["b1"][l])
        w2 = np.asarray(inp["W2"][l], np.float32)
        w[f"W2{l}"] = _bf16(w2.reshape(NFC, 128, NKC, 128).transpose(2, 1, 0, 3))
        w[f"b2{l}"] = _btile(inp["b2"][l])
        w[f"ln1s{l}"] = _btile(inp["ln1_s"][l])
        w[f"ln1b{l}"] = _btile(inp["ln1_b"][l])
        w[f"ln2s{l}"] = _btile(inp["ln2_s"][l])
        w[f"ln2b{l}"] = _btile(inp["ln2_b"][l])
    w["lnfs"] = _btile(inp["lnf_s"])
    w["lnfb"] = _btile(inp["lnf_b"])
    w["headW"] = _wtile(inp["head_W"])
    w["headB"] = _f32(np.asarray(inp["head_b"], np.float32)[:, None])
    w["c1W"] = _bf16(inp["c1_W"])
    w["c1B"] = _btile(inp["c1_b"])
    w["c2W"] = _wtile(inp["c2_W"])
    w["c2B"] = _f32(np.asarray(inp["c2_b"], np.float32)[:, None])
    w["c3W"] = _bf16(inp["c3_W"])
    w["c3B"] = _f32(np.asarray(inp["c3_b"], np.float32)[:, None])
    return w


def kernel(**inputs):
    global _BUILT, LAST_RESULT
    if _BUILT is None:
        _BUILT = _build()
    nc, names = _BUILT

    w = _host_weights(inputs)
    x = np.asarray(inputs["x"], np.float32)  # [4, 3072, 21]

    # precomputed key-window map (global gathered order) and masks per parity
    wk = np.concatenate([np.repeat(np.arange(16) * 2, C),
                         np.repeat(np.arange(16) * 2 + 1, C)])  # [672]
    in_maps = []
    for core in range(8):
        b, parity = core // 2, core % 2
        wins = np.arange(16) * 2 + parity
        xb = x[b]  # [3072, 21]
        xl = np.empty((P, SL), np.float32)
        for i, wn in enumerate(wins):
            xl[:, i * C:(i + 1) * C] = xb[wn * P:(wn + 1) * P, :]
        wq = np.repeat(wins, C)
        mask = (wk[:, None] <= wq[None, :]).astype(np.float32)  # [672, 336]
        mask3 = mask.reshape(6, 112, SL).transpose(1, 0, 2)     # [112, 6, 336]
        m = dict(w)
        m["xfull"] = _f32(xb.T)
        m["xloc"] = _bf16(xl)
        m["maskM"] = _bf16(mask3)
        in_maps.append(m)

    res = run_bass_kernel_spmd(nc, in_maps, core_ids=list(range(8)))
    LAST_RESULT = res
    logits = np.stack(
        [res.results[2 * b]["out"].reshape(2).astype(np.float32) for b in range(B)])
    return logits


# revision 19
# speedup vs baseline: 1.0361x; 1.0361x over previous
"""Trainium2 Bass kernel for nn_Model_29592324670139 (dense transformer).

Sharding: 8 cores = 4 pairs. Pair b handles batch item b; within a pair the
672-token sequence (21 vars x 32 windows, window-major order) is split by
window parity (rank0 = even windows, rank1 = odd windows), 336 tokens each.
Per layer, each core projects Q/K/V for its tokens; K/V are AllGathered
within the pair; attention/FFN/LN run on local tokens. The final pooled
feature sum is AllGathered, and head+MLP run redundantly per pair.

Activations are feature-major ([d, token] with d on partitions). Matmuls in
bf16 with fp32 PSUM accumulation; softmax/LN statistics in fp32.

Self-contained: hardcodes all shapes; only needs numpy/ml_dtypes/concourse.
"""

import numpy as np
import ml_dtypes

import concourse.bass as bass
import concourse.tile as tile
from concourse import bacc, mybir
from concourse.bass import ts, ds
from concourse.bass_utils import run_bass_kernel_spmd

F32 = mybir.dt.float32
BF16 = mybir.dt.bfloat16
F32R = mybir.dt.float32r
AX = mybir.AluOpType
AF = mybir.ActivationFunctionType
XL = mybir.AxisListType

B, L, C = 4, 3072, 21
P, OUT, D, H, NL, DFF = 96, 96, 1024, 16, 2, 4096
NW = 32          # windows
SL = 336         # local tokens per core
S = 672          # full sequence
HD = 64          # head dim
NKC = D // 128   # 8 k-chunks of d_model
NFC = DFF // 128  # 32 chunks of d_ff

REPLICA_GROUPS = [[0, 1], [2, 3], [4, 5], [6, 7]]

_BUILT = None  # cached (nc, input_names)

LAST_RESULT = None  # stash of the last BassKernelResults (for test harness)


# ----------------------------------------------------------------------------
# device program
# ----------------------------------------------------------------------------

def _build():
    nc = bacc.Bacc("TRN2", target_bir_lowering=False, debug=False,
                   enable_asserts=False, num_devices=8)

    t = {}

    def din(name, shape, dt):
        t[name] = nc.dram_tensor(name, list(shape), dt, kind="ExternalInput").ap()

    # per-core data
    din("xfull", (C, L), F32)
    din("xloc", (P, SL), BF16)
    din("maskM", (112, 6, SL), BF16)
    # embedding
    din("embW", (P, D), BF16)
    din("embWsumN", (128, NKC), F32)
    din("embB", (128, NKC), F32)
    for l in range(NL):
        for w, bvec in (("Wq", "bq"), ("Wk", "bk"), ("Wo", "bo")):
            din(f"{w}{l}", (128, NKC, D), BF16)      # [p, kc, n] pre-arranged
            din(f"{bvec}{l}", (128, NKC), F32)
        din(f"Wv{l}", (128, NKC, D), BF16)
        din(f"bv{l}", (1, D), BF16)
        din(f"W1{l}", (4, 128, NKC, 1024), BF16)     # quarters [j, p, kc, n]
        din(f"b1{l}", (128, NFC), F32)
        din(f"W2{l}", (NKC, 128, NFC, 128), BF16)    # per-oc [oc, p, kc, c]
        din(f"b2{l}", (128, NKC), F32)
        for v in ("ln1s", "ln1b", "ln2s", "ln2b"):
            din(f"{v}{l}", (128, NKC), F32)
    din("lnfs", (128, NKC), F32)
    din("lnfb", (128, NKC), F32)
    din("headW", (128, NKC, OUT), BF16)
    din("headB", (OUT, 1), F32)
    din("c1W", (OUT, 256), BF16)
    din("c1B", (128, 2), F32)
    din("c2W", (128, 2, 64), BF16)
    din("c2B", (64, 1), F32)
    din("c3W", (64, 2), BF16)
    din("c3B", (2, 1), F32)

    out_dram = nc.dram_tensor("out", [2, 1], F32, kind="ExternalOutput").ap()

    with tile.TileContext(nc) as tc:
        _emit(tc, t, out_dram)

    nc.compile()
    return nc, set(t.keys())


def _emit(tc, t, out_dram):
    from contextlib import ExitStack
    nc = tc.nc
    ctx = ExitStack()

    # ---------------- pools ----------------
    constp = ctx.enter_context(tc.tile_pool(name="constp", bufs=1))
    wpool = ctx.enter_context(tc.tile_pool(name="wpool", bufs=2))
    actp = ctx.enter_context(tc.tile_pool(name="actp", bufs=1))
    esbp = ctx.enter_context(tc.tile_pool(name="esbp", bufs=6))
    lnp = ctx.enter_context(tc.tile_pool(name="lnp", bufs=1))
    sqp = ctx.enter_context(tc.tile_pool(name="sqp", bufs=3))
    recp = ctx.enter_context(tc.tile_pool(name="recp", bufs=2))
    w2p = ctx.enter_context(tc.tile_pool(name="w2p", bufs=3))
    bcp = ctx.enter_context(tc.tile_pool(name="bcp", bufs=1))
    dramp = ctx.enter_context(tc.tile_pool(name="dramp", bufs=1, space="DRAM"))
    psS = ctx.enter_context(tc.tile_pool(name="psS", bufs=2, space="PSUM"))
    psAV = ctx.enter_context(tc.tile_pool(name="psAV", bufs=2, space="PSUM"))
    psMM = ctx.enter_context(tc.tile_pool(name="psMM", bufs=2, space="PSUM"))
    psST = ctx.enter_context(tc.tile_pool(name="psST", bufs=2, space="PSUM"))

    def single(shape, dt, name, **kw):
        tl, free = tc.tile(shape, dt, name=name, **kw)
        ctx.callback(free)
        return tl

    # ---------------- hot-path loads first (stage-0 + embedding inputs) ----
    xloc_sb = constp.tile([P, SL], BF16, name="xloc_sb", tag="xloc_sb")
    nc.sync.dma_start(out=xloc_sb[:], in_=t["xloc"][:])
    embW_sb = constp.tile([P, D], BF16, name="embW_sb", tag="embW_sb")
    nc.sync.dma_start(out=embW_sb[:], in_=t["embW"][:])

    # ---------------- constants / small tensors ----------------
    def vec_to_sb(name, dim):
        # pre-arranged [128, dim//128] f32 dram -> sbuf
        nchunk = dim // 128
        sb = constp.tile([128, nchunk], F32, name=f"sb_{name}", tag=f"sb_{name}")
        nc.sync.dma_start(out=sb[:], in_=t[name][:])
        return sb

    sb_embWsumN = vec_to_sb("embWsumN", D)
    sb_embB = vec_to_sb("embB", D)
    bias_sb = {}
    for l in range(NL):
        for v in ("bq", "bk", "bo", "b2", "ln1s", "ln1b", "ln2s", "ln2b"):
            bias_sb[f"{v}{l}"] = vec_to_sb(f"{v}{l}", D)
        bias_sb[f"b1{l}"] = vec_to_sb(f"b1{l}", DFF)
    sb_lnfs = vec_to_sb("lnfs", D)
    sb_lnfb = vec_to_sb("lnfb", D)

    # bv as [1, 1024] rows for partition_broadcast
    sb_bv = {}
    for l in range(NL):
        bv1 = constp.tile([1, D], BF16, name=f"bv1_{l}", tag=f"bv1_{l}")
        nc.sync.dma_start(out=bv1[:], in_=t[f"bv{l}"][:])
        sb_bv[l] = bv1

    sb_headB = constp.tile([OUT, 1], F32, name="sb_headB", tag="sb_headB")
    nc.sync.dma_start(out=sb_headB[:], in_=t["headB"][:])
    sb_c1B = constp.tile([128, 2], F32, name="sb_c1B", tag="sb_c1B")
    nc.sync.dma_start(out=sb_c1B[:], in_=t["c1B"][:])
    sb_c2B = constp.tile([64, 1], F32, name="sb_c2B", tag="sb_c2B")
    nc.sync.dma_start(out=sb_c2B[:], in_=t["c2B"][:])
    sb_c3B = constp.tile([2, 1], F32, name="sb_c3B", tag="sb_c3B")
    nc.sync.dma_start(out=sb_c3B[:], in_=t["c3B"][:])

    sb_c1W = constp.tile([OUT, 256], BF16, name="sb_c1W", tag="sb_c1W")
    nc.sync.dma_start(out=sb_c1W[:], in_=t["c1W"][:])
    sb_c2W = constp.tile([128, 2, 64], BF16, name="sb_c2W", tag="sb_c2W")
    nc.sync.dma_start(out=sb_c2W[:], in_=t["c2W"][:])
    sb_c3W = constp.tile([64, 2], BF16, name="sb_c3W", tag="sb_c3W")
    nc.sync.dma_start(out=sb_c3W[:], in_=t["c3W"][:])
    sb_headW = constp.tile([128, NKC, OUT], BF16, name="sb_headW", tag="sb_headW")
    nc.sync.dma_start(out=sb_headW[:], in_=t["headW"][:])

    sb_mask = constp.tile([112, 6, SL], BF16, name="sb_mask", tag="sb_mask")
    nc.sync.dma_start(out=sb_mask[:], in_=t["maskM"][:])

    ones_bf = constp.tile([128, 1], BF16, name="ones_bf", tag="ones_bf")
    nc.vector.memset(ones_bf[:], 1.0)
    eps6_sb = constp.tile([C, 1], F32, name="eps6_sb", tag="eps6_sb")
    nc.vector.memset(eps6_sb[:], 1e-6)
    eps5_sb = constp.tile([1, 1], F32, name="eps5_sb", tag="eps5_sb")
    nc.vector.memset(eps5_sb[:], 1e-5)
    zero_sb = constp.tile([128, 1], F32, name="zero_sb", tag="zero_sb")
    nc.vector.memset(zero_sb[:], 0.0)

    # ---------------- stage 0: instance norm stats ----------------
    st6 = constp.tile([C, 6, 6], F32, name="st6", tag="st6")
    xfp = ctx.enter_context(tc.tile_pool(name="xfp", bufs=3))
    for i in range(6):
        xfc = xfp.tile([C, 512], F32, name="xfc", tag="xfc")
        nc.sync.dma_start(out=xfc[:], in_=t["xfull"][:, ts(i, 512)])
        nc.vector.bn_stats(out=st6[:, i, :], in_=xfc[:])
    mv = constp.tile([C, 2], F32, name="mv", tag="mv")
    nc.vector.bn_aggr(out=mv[:], in_=st6[:])
    std21 = constp.tile([C, 1], F32, name="std21", tag="std21")
    nc.scalar.activation(out=std21[:], in_=mv[:, 1:2], func=AF.Sqrt, bias=eps6_sb[:])
    stat2 = constp.tile([C, 2], F32, name="stat2", tag="stat2")
    nc.vector.reciprocal(out=stat2[:, 0:1], in_=std21[:])
    nc.vector.tensor_mul(stat2[:, 1:2], mv[:, 0:1], stat2[:, 0:1])

    stat_dram = dramp.tile([C, 2], F32, name="stat_dram", tag="stat_dram")
    nc.sync.dma_start(out=stat_dram[:], in_=stat2[:])
    # per-token [1, 336] vectors: token t -> channel c = t % 21
    rstd_tok = constp.tile([1, SL], F32, name="rstd_tok", tag="rstd_tok")
    nc.sync.dma_start(
        out=rstd_tok[:].rearrange("p (n c) -> p n c", c=C),
        in_=bass.AP(tensor=stat_dram[:].tensor, offset=stat_dram[:].offset,
                    ap=[[0, 16], [2, C]]))
    mrs_tok = constp.tile([1, SL], F32, name="mrs_tok", tag="mrs_tok")
    nc.sync.dma_start(
        out=mrs_tok[:].rearrange("p (n c) -> p n c", c=C),
        in_=bass.AP(tensor=stat_dram[:].tensor, offset=stat_dram[:].offset + 1,
                    ap=[[0, 16], [2, C]]))
    rt_b = constp.tile([128, SL], F32, name="rt_b", tag="rt_b")
    nc.gpsimd.partition_broadcast(out_ap=rt_b[:], in_ap=rstd_tok[:])
    mrs_b = constp.tile([128, SL], F32, name="mrs_b", tag="mrs_b")
    nc.gpsimd.partition_broadcast(out_ap=mrs_b[:], in_ap=mrs_tok[:])

    # ---------------- stage 1: embedding ----------------
    h_f32 = single([128, NKC, SL], F32, "h_f32")
    h_bf = single([128, NKC, SL], BF16, "h_bf")
    d1 = single([128, NKC, SL], F32, "d1")

    for c8 in range(NKC):
        pse = psMM.tile([128, SL], F32, name="pse", tag="mm")
        nc.tensor.matmul(pse[:], lhsT=embW_sb[:, ts(c8, 128)], rhs=xloc_sb[:],
                         start=True, stop=True)
        nc.vector.tensor_mul(d1[:, c8, :], pse[:], rt_b[:])
        nc.vector.scalar_tensor_tensor(
            out=d1[:, c8, :], in0=mrs_b[:], scalar=sb_embWsumN[:, c8:c8 + 1],
            in1=d1[:, c8, :], op0=AX.mult, op1=AX.add)
        nc.scalar.activation(out=h_f32[:, c8, :], in_=d1[:, c8, :],
                             func=AF.Identity, bias=sb_embB[:, c8:c8 + 1])
        nc.scalar.activation(out=h_bf[:, c8, :], in_=d1[:, c8, :],
                             func=AF.Identity, bias=sb_embB[:, c8:c8 + 1])

    # ---------------- helpers ----------------
    def load_w(src, n, tag="w"):
        """Load a pre-arranged [128, NKC, n] bf16 weight tile."""
        w = wpool.tile([128, NKC, n], BF16, name="w_t", tag=tag)
        nc.sync.dma_start(out=w[:], in_=src)
        return w

    def ln_stats(src):
        """Returns (mean1, r1, r_b) for feature-major LN over src (f32)."""
        ps_sum = psST.tile([1, SL], F32, name="ps_sum", tag="st")
        ps_sq = psST.tile([1, SL], F32, name="ps_sq", tag="st")
        sqs = []
        for c8 in range(NKC):
            nc.scalar.activation(out=h_bf[:, c8, :], in_=src[:, c8, :],
                                 func=AF.Identity, bias=zero_sb[:])
            sq_c = sqp.tile([128, SL], BF16, name="sq_c", tag="sq_c")
            nc.scalar.activation(out=sq_c[:], in_=src[:, c8, :], func=AF.Square)
            sqs.append(sq_c)
        for c8 in range(NKC):
            nc.tensor.matmul(ps_sum[:], lhsT=ones_bf[:],
                             rhs=h_bf[:, c8, :],
                             start=(c8 == 0), stop=(c8 == NKC - 1))
        for c8 in range(NKC):
            nc.tensor.matmul(ps_sq[:], lhsT=ones_bf[:],
                             rhs=sqs[c8][:],
                             start=(c8 == 0), stop=(c8 == NKC - 1))
        mean1 = lnp.tile([1, SL], F32, name="mean1", tag="mean1")
        nc.scalar.activation(out=mean1[:], in_=ps_sum[:], func=AF.Copy,
                             scale=1.0 / D)
        ms1 = lnp.tile([1, SL], F32, name="ms1", tag="ms1")
        nc.vector.tensor_mul(ms1[:], mean1[:], mean1[:])
        var1 = lnp.tile([1, SL], F32, name="var1", tag="var1")
        nc.vector.scalar_tensor_tensor(out=var1[:], in0=ps_sq[:], scalar=1.0 / D,
                                       in1=ms1[:], op0=AX.mult, op1=AX.subtract)
        std1 = lnp.tile([1, SL], F32, name="std1", tag="std1")
        nc.scalar.activation(out=std1[:], in_=var1[:], func=AF.Sqrt, bias=eps5_sb[:])
        r1 = lnp.tile([1, SL], F32, name="r1", tag="r1")
        nc.vector.reciprocal(out=r1[:], in_=std1[:])
        r_b = bcp.tile([128, SL], F32, name="r_b", tag="r_b")
        nc.gpsimd.partition_broadcast(out_ap=r_b[:], in_ap=r1[:])
        return mean1, r1, r_b

    def ln(s_sb, b_sb, write_bf, src):
        """Feature-major layernorm: reads src (f32), writes h_f32 (+h_bf)."""
        mean1, r1, r_b = ln_stats(src)
        mean_b = bcp.tile([128, SL], F32, name="mean_b", tag="mean_b")
        nc.gpsimd.partition_broadcast(out_ap=mean_b[:], in_ap=mean1[:])
        for c8 in range(NKC):
            nc.vector.tensor_sub(d1[:, c8, :], src[:, c8, :], mean_b[:])
            nc.vector.scalar_tensor_tensor(
                out=d1[:, c8, :], in0=d1[:, c8, :], scalar=s_sb[:, c8:c8 + 1],
                in1=r_b[:], op0=AX.mult, op1=AX.mult)
            nc.scalar.activation(out=h_f32[:, c8, :], in_=d1[:, c8, :],
                                 func=AF.Identity, bias=b_sb[:, c8:c8 + 1])
            if write_bf:
                nc.scalar.activation(out=h_bf[:, c8, :], in_=d1[:, c8, :],
                                     func=AF.Identity, bias=b_sb[:, c8:c8 + 1])

    # ---------------- transformer layers ----------------
    for l in range(NL):
        wk = load_w(t[f"Wk{l}"][:], D)
        wv = load_w(t[f"Wv{l}"][:], D)
        wq = load_w(t[f"Wq{l}"][:], D)
        KREG = 128 * NKC * SL
        VREG = SL * D

        # K feature-major projection -> bf16, bounce, AllGather (first)
        k_sb = actp.tile([128, NKC, SL], BF16, name="k_sb", tag="k_sb")
        bsb = bias_sb[f"bk{l}"]
        for oc in range(NKC):
            psp = psMM.tile([128, SL], F32, name="psp", tag="mm")
            for kc in range(NKC):
                nc.tensor.matmul(psp[:], lhsT=wk[:, kc, ts(oc, 128)],
                                 rhs=h_bf[:, kc, :],
                                 start=(kc == 0), stop=(kc == NKC - 1))
            nc.scalar.activation(out=k_sb[:, oc, :], in_=psp[:],
                                 func=AF.Identity, bias=bsb[:, oc:oc + 1])
        kbnc_in = dramp.tile([KREG], BF16, name=f"kbnc_in{l}", tag=f"kbnc_in{l}")
        nc.sync.dma_start(
            out=kbnc_in[ds(0, KREG)].rearrange("(kc p tk) -> p kc tk",
                                               p=128, tk=SL),
            in_=k_sb[:])
        kbnc_out = single([2 * KREG], BF16, f"kbnc_out{l}", space="DRAM",
                          addr_space="Shared")
        nc.gpsimd.collective_compute(
            "AllGather", AX.bypass, replica_groups=REPLICA_GROUPS,
            ins=[kbnc_in[:]], outs=[kbnc_out[:]])

        # V token-major projection  [112, 3, 1024], bounce, AllGather
        bv_b = actp.tile([112, D], BF16, name="bv_b", tag="bv_b")
        nc.gpsimd.partition_broadcast(out_ap=bv_b[:], in_ap=sb_bv[l][:])
        v_sb = actp.tile([112, 3, D], BF16, name="v_sb", tag="v_sb")
        for tc3 in range(3):
            for nh in range(2):
                psv = psMM.tile([112, 512], F32, name="psv", tag="mm")
                for kc in range(NKC):
                    nc.tensor.matmul(psv[:], lhsT=h_bf[:, kc, ds(tc3 * 112, 112)],
                                     rhs=wv[:, kc, ts(nh, 512)],
                                     start=(kc == 0), stop=(kc == NKC - 1))
                nc.vector.tensor_add(
                    v_sb[:, tc3, ts(nh, 512)], psv[:], bv_b[:, ts(nh, 512)])
        vbnc_in = dramp.tile([VREG], BF16, name=f"vbnc_in{l}", tag=f"vbnc_in{l}")
        nc.sync.dma_start(
            out=vbnc_in[ds(0, VREG)].rearrange("(t3 p he) -> p t3 he",
                                               p=112, he=D),
            in_=v_sb[:])
        vbnc_out = single([2 * VREG], BF16, f"vbnc_out{l}", space="DRAM",
                          addr_space="Shared")
        nc.gpsimd.collective_compute(
            "AllGather", AX.bypass, replica_groups=REPLICA_GROUPS,
            ins=[vbnc_in[:]], outs=[vbnc_out[:]])

        # Q feature-major projection (overlaps the AllGathers)
        q_sb = actp.tile([128, NKC, SL], BF16, name="q_sb", tag="q_sb")
        bsb = bias_sb[f"bq{l}"]
        for oc in range(NKC):
            psp = psMM.tile([128, SL], F32, name="psp", tag="mm")
            for kc in range(NKC):
                nc.tensor.matmul(psp[:], lhsT=wq[:, kc, ts(oc, 128)],
                                 rhs=h_bf[:, kc, :],
                                 start=(kc == 0), stop=(kc == NKC - 1))
            nc.scalar.activation(out=q_sb[:, oc, :], in_=psp[:],
                                 func=AF.Identity, bias=bsb[:, oc:oc + 1])

        k_full = actp.tile([128, NKC, S], BF16, name="k_full", tag="k_full")
        for r in range(2):
            nc.sync.dma_start(
                out=k_full[:, :, ds(r * SL, SL)],
                in_=kbnc_out[ds(r * KREG, KREG)].rearrange(
                    "(kc p tk) -> p kc tk", p=128, tk=SL))
        v_full = actp.tile([112, 6, H, HD + 1], BF16, name="v_full", tag="v_full")
        nc.vector.memset(v_full[:, :, :, HD:HD + 1], 1.0)
        for c6 in range(6):
            off = (c6 // 3) * VREG + (c6 % 3) * 112 * D
            nc.sync.dma_start(
                out=v_full[:, c6, :, 0:HD],
                in_=vbnc_out[ds(off, 112 * D)].rearrange(
                    "(p hh e) -> p hh e", p=112, e=HD))

        # attention
        att_sb = actp.tile([128, NKC, SL], BF16, name="att_sb", tag="att_sb")
        stage_odd = actp.tile([64, NKC, SL], BF16, name="stage_odd", tag="stage_odd")
        for hh in range(H):
            hb2 = 64 * (hh % 2)
            hc = hh // 2
            esbs = []
            for cc in range(6):
                pss = psS.tile([112, SL], F32, name="pss", tag="s")
                nc.tensor.matmul(pss[:],
                                 lhsT=k_full[ds(hb2, 64), hc, ts(cc, 112)],
                                 rhs=q_sb[ds(hb2, 64), hc, :],
                                 start=True, stop=True)
                esb = esbp.tile([112, SL], BF16, name="esb", tag="esb")
                nc.scalar.activation(out=esb[:], in_=pss[:], func=AF.Exp)
                nc.vector.tensor_mul(esb[:], esb[:], sb_mask[:, cc, :])
                esbs.append(esb)
            psa = psAV.tile([HD + 1, SL], F32, name="psa", tag="av")
            for cc in range(6):
                nc.tensor.matmul(psa[:], lhsT=v_full[:, cc, hh, :],
                                 rhs=esbs[cc][:],
                                 start=(cc == 0), stop=(cc == 5))
            rec = recp.tile([1, SL], F32, name="rec", tag="rec")
            nc.vector.reciprocal(out=rec[:], in_=psa[ds(HD, 1), :])
            rb = recp.tile([64, SL], F32, name="rb", tag="rb")
            nc.gpsimd.partition_broadcast(out_ap=rb[:], in_ap=rec[:])
            dst = att_sb[0:64, hc, :] if hh % 2 == 0 else stage_odd[:, hc, :]
            nc.vector.tensor_mul(dst, psa[0:HD, :], rb[:])
        nc.sync.dma_start(out=att_sb[ds(64, 64), :, :], in_=stage_odd[:])

        # out-proj + residual
        wo = load_w(t[f"Wo{l}"][:], D)
        bo_sb = bias_sb[f"bo{l}"]
        for oc in range(NKC):
            pso = psMM.tile([128, SL], F32, name="pso", tag="mm")
            for kc in range(NKC):
                nc.tensor.matmul(pso[:], lhsT=wo[:, kc, ts(oc, 128)],
                                 rhs=att_sb[:, kc, :],
                                 start=(kc == 0), stop=(kc == NKC - 1))
            nc.vector.scalar_tensor_tensor(
                out=d1[:, oc, :], in0=pso[:], scalar=bo_sb[:, oc:oc + 1],
                in1=h_f32[:, oc, :], op0=AX.add, op1=AX.add)

        ln(bias_sb[f"ln1s{l}"], bias_sb[f"ln1b{l}"], write_bf=True, src=d1)

        # FFN
        w1q = [load_w(t[f"W1{l}"][j], 1024) for j in range(4)]
        g_sb = actp.tile([128, NFC, SL], BF16, name="g_sb", tag="g_sb")
        b1_sb = bias_sb[f"b1{l}"]
        for fc in range(NFC):
            w1 = w1q[fc // 8]
            psf = psMM.tile([128, SL], F32, name="psf", tag="mm")
            for kc in range(NKC):
                nc.tensor.matmul(psf[:], lhsT=w1[:, kc, ts(fc % 8, 128)],
                                 rhs=h_bf[:, kc, :],
                                 start=(kc == 0), stop=(kc == NKC - 1))
            nc.scalar.activation(out=g_sb[:, fc, :], in_=psf[:], func=AF.Gelu,
                                 bias=b1_sb[:, fc:fc + 1])
        b2_sb = bias_sb[f"b2{l}"]
        for oc in range(NKC):
            w2 = w2p.tile([128, NFC, 128], BF16, name="w2oc", tag="w2oc")
            nc.sync.dma_start(out=w2[:], in_=t[f"W2{l}"][oc])
            psy = psMM.tile([128, SL], F32, name="psy", tag="mm")
            for kc in range(NFC):
                nc.tensor.matmul(psy[:], lhsT=w2[:, kc, ts(0, 128)],
                                 rhs=g_sb[:, kc, :],
                                 start=(kc == 0), stop=(kc == NFC - 1))
            nc.vector.scalar_tensor_tensor(
                out=d1[:, oc, :], in0=psy[:], scalar=b2_sb[:, oc:oc + 1],
                in1=h_f32[:, oc, :], op0=AX.add, op1=AX.add)

        ln(bias_sb[f"ln2s{l}"], bias_sb[f"ln2b{l}"], write_bf=(l < NL - 1), src=d1)

    # ---------------- final: fused LN_f + pooling, head, MLP ----------------
    # pooled[d] = (1/672) * s_d * (sum_t h[d,t]*r_t - sum_t m_t*r_t) + b_d
    mean1, r1, r_b = ln_stats(h_f32)
    mr1 = lnp.tile([1, SL], F32, name="mr1", tag="mr1")
    nc.vector.tensor_mul(mr1[:], mean1[:], r1[:])
    csc = lnp.tile([1, 1], F32, name="csc", tag="csc")
    nc.vector.reduce_sum(out=csc[:], in_=mr1[:], axis=XL.X)
    hsum = constp.tile([128, NKC], F32, name="hsum", tag="hsum")
    for c8 in range(NKC):
        nc.vector.scalar_tensor_tensor(
            out=d1[:, c8, :], in0=h_f32[:, c8, :], scalar=1.0, in1=r_b[:],
            op0=AX.mult, op1=AX.mult, accum_out=hsum[:, c8:c8 + 1])
    FD = D + 8  # 32B-aligned block: 1024 sums + csc + pad
    fin_in = dramp.tile([FD], F32, name="fin_in", tag="fin_in")
    nc.sync.dma_start(out=fin_in[ds(0, D)].rearrange("(kc p) -> p kc", p=128),
                      in_=hsum[:])
    nc.sync.dma_start(out=fin_in[ds(D, 1)], in_=csc[:])
    fin_out = single([2 * FD], F32, "fin_out", space="DRAM",
                     addr_space="Shared")
    nc.gpsimd.collective_compute(
        "AllGather", AX.bypass, replica_groups=REPLICA_GROUPS,
        ins=[fin_in[:]], outs=[fin_out[:]])
    ffx = constp.tile([128, NKC, 2], F32, name="ffx", tag="ffx")
    for r in range(2):
        nc.sync.dma_start(
            out=ffx[:, :, r],
            in_=fin_out[ds(r * FD, D)].rearrange("(kc p) -> p kc", p=128))
    csc2 = constp.tile([1, 2], F32, name="csc2", tag="csc2")
    for r in range(2):
        nc.sync.dma_start(out=csc2[:, r:r + 1], in_=fin_out[ds(r * FD + D, 1)])
    cst = constp.tile([1, 1], F32, name="cst", tag="cst")
    nc.vector.tensor_add(cst[:], csc2[:, 0:1], csc2[:, 1:2])
    c_b = constp.tile([128, 1], F32, name="c_b", tag="c_b")
    nc.gpsimd.partition_broadcast(out_ap=c_b[:], in_ap=cst[:])
    hb2t = constp.tile([128, NKC], F32, name="hb2t", tag="hb2t")
    nc.vector.tensor_add(hb2t[:], ffx[:, :, 0], ffx[:, :, 1])
    u2 = constp.tile([128, NKC], F32, name="u2", tag="u2")
    nc.vector.tensor_scalar(out=u2[:], in0=hb2t[:], scalar1=c_b[:],
                            scalar2=1.0 / S, op0=AX.subtract, op1=AX.mult)
    u3 = constp.tile([128, NKC], F32, name="u3", tag="u3")
    nc.vector.tensor_mul(u3[:], u2[:], sb_lnfs[:])
    hbar_bf = constp.tile([128, NKC], BF16, name="hbar_bf", tag="hbar_bf")
    nc.vector.tensor_add(hbar_bf[:], u3[:], sb_lnfb[:])

    psh = psMM.tile([OUT, 1], F32, name="psh", tag="mm")
    for kc in range(NKC):
        nc.tensor.matmul(psh[:], lhsT=sb_headW[:, kc, :],
                         rhs=hbar_bf[:, kc:kc + 1],
                         start=(kc == 0), stop=(kc == NKC - 1))
    feat_bf = constp.tile([OUT, 1], BF16, name="feat_bf", tag="feat_bf")
    nc.scalar.activation(out=feat_bf[:], in_=psh[:], func=AF.Identity,
                         bias=sb_headB[:])

    z1_bf = constp.tile([128, 2], BF16, name="z1_bf", tag="z1_bf")
    for i2 in range(2):
        psc = psMM.tile([128, 1], F32, name="psc", tag="mm")
        nc.tensor.matmul(psc[:], lhsT=sb_c1W[:, ts(i2, 128)], rhs=feat_bf[:],
                         start=True, stop=True)
        nc.scalar.activation(out=z1_bf[:, i2:i2 + 1], in_=psc[:], func=AF.Relu,
                             bias=sb_c1B[:, i2:i2 + 1])
    psc2 = psMM.tile([64, 1], F32, name="psc2", tag="mm")
    for kc in range(2):
        nc.tensor.matmul(psc2[:], lhsT=sb_c2W[:, kc, :], rhs=z1_bf[:, kc:kc + 1],
                         start=(kc == 0), stop=(kc == 1))
    z2_bf = constp.tile([64, 1], BF16, name="z2_bf", tag="z2_bf")
    nc.scalar.activation(out=z2_bf[:], in_=psc2[:], func=AF.Relu, bias=sb_c2B[:])
    psc3 = psMM.tile([2, 1], F32, name="psc3", tag="mm")
    nc.tensor.matmul(psc3[:], lhsT=sb_c3W[:], rhs=z2_bf[:], start=True, stop=True)
    out_sb = constp.tile([2, 1], F32, name="out_sb", tag="out_sb")
    nc.scalar.activation(out=out_sb[:], in_=psc3[:], func=AF.Identity,
                         bias=sb_c3B[:])
    nc.sync.dma_start(out=out_dram[:], in_=out_sb[:])
    ctx.close()


# ----------------------------------------------------------------------------
# host side
# ----------------------------------------------------------------------------

def _bf16(x):
    return np.ascontiguousarray(np.asarray(x, dtype=np.float32)).astype(
        ml_dtypes.bfloat16)


def _f32(x):
    return np.ascontiguousarray(np.asarray(x, dtype=np.float32))


def _wtile(a):
    # [D_in, N] -> [128, D_in//128, N] (p, kc, n)
    a = np.asarray(a, np.float32)
    din, n = a.shape
    return _bf16(a.reshape(din // 128, 128, n).transpose(1, 0, 2))


def _btile(a, p=128):
    # [dim] -> [p, dim//p]
    a = np.asarray(a, np.float32)
    return _f32(a.reshape(-1, p).T)


def _host_weights(inp):
    w = {}
    w["embW"] = _bf16(inp["emb_W"])
    w["embWsumN"] = _btile(-np.asarray(inp["emb_W"], np.float32).sum(0))
    w["embB"] = _btile(inp["emb_b"])
    for l in range(NL):
        w[f"Wq{l}"] = _wtile(np.asarray(inp["Wq"][l], np.float32) * 0.125)
        w[f"bq{l}"] = _btile(np.asarray(inp["bq"][l], np.float32) * 0.125)
        w[f"Wk{l}"] = _wtile(inp["Wk"][l])
        w[f"bk{l}"] = _btile(inp["bk"][l])
        w[f"Wv{l}"] = _wtile(inp["Wv"][l])
        w[f"bv{l}"] = _bf16(np.asarray(inp["bv"][l], np.float32)[None, :])
        w[f"Wo{l}"] = _wtile(inp["Wo"][l])
        w[f"bo{l}"] = _btile(inp["bo"][l])
        w1 = np.asarray(inp["W1"][l], np.float32)
        w[f"W1{l}"] = _bf16(w1.reshape(NKC, 128, 4, 1024).transpose(2, 1, 0, 3))
        w[f"b1{l}"] = _btile(inp["b1"][l])
        w2 = np.asarray(inp["W2"][l], np.float32)
        w[f"W2{l}"] = _bf16(w2.reshape(NFC, 128, NKC, 128).transpose(2, 1, 0, 3))
        w[f"b2{l}"] = _btile(inp["b2"][l])
        w[f"ln1s{l}"] = _btile(inp["ln1_s"][l])
        w[f"ln1b{l}"] = _btile(inp["ln1_b"][l])
        w[f"ln2s{l}"] = _btile(inp["ln2_s"][l])
        w[f"ln2b{l}"] = _btile(inp["ln2_b"][l])
    w["lnfs"] = _btile(inp["lnf_s"])
    w["lnfb"] = _btile(inp["lnf_b"])
    w["headW"] = _wtile(inp["head_W"])
    w["headB"] = _f32(np.asarray(inp["head_b"], np.float32)[:, None])
    w["c1W"] = _bf16(inp["c1_W"])
    w["c1B"] = _btile(inp["c1_b"])
    w["c2W"] = _wtile(inp["c2_W"])
    w["c2B"] = _f32(np.asarray(inp["c2_b"], np.float32)[:, None])
    w["c3W"] = _bf16(inp["c3_W"])
    w["c3B"] = _f32(np.asarray(inp["c3_b"], np.float32)[:, None])
    return w


def kernel(**inputs):
    global _BUILT, LAST_RESULT
    if _BUILT is None:
        _BUILT = _build()
    nc, names = _BUILT

    w = _host_weights(inputs)
    x = np.asarray(inputs["x"], np.float32)  # [4, 3072, 21]

    # precomputed key-window map (global gathered order) and masks per parity
    wk = np.concatenate([np.repeat(np.arange(16) * 2, C),
                         np.repeat(np.arange(16) * 2 + 1, C)])  # [672]
    in_maps = []
    for core in range(8):
        b, parity = core // 2, core % 2
        wins = np.arange(16) * 2 + parity
        xb = x[b]  # [3072, 21]
        xl = np.empty((P, SL), np.float32)
        for i, wn in enumerate(wins):
            xl[:, i * C:(i + 1) * C] = xb[wn * P:(wn + 1) * P, :]
        wq = np.repeat(wins, C)
        mask = (wk[:, None] <= wq[None, :]).astype(np.float32)  # [672, 336]
        mask3 = mask.reshape(6, 112, SL).transpose(1, 0, 2)     # [112, 6, 336]
        m = dict(w)
        m["xfull"] = _f32(xb.T)
        m["xloc"] = _bf16(xl)
        m["maskM"] = _bf16(mask3)
        in_maps.append(m)

    res = run_bass_kernel_spmd(nc, in_maps, core_ids=list(range(8)))
    LAST_RESULT = res
    logits = np.stack(
        [res.results[2 * b]["out"].reshape(2).astype(np.float32) for b in range(B)])
    return logits
